# revision 8
# baseline (speedup 1.0000x reference)
"""Trainium2 Bass kernel for CustomMHA (fused qkv proj + tanh-clipped softmax attn).

Problem (hardcoded shapes):
  query/key/value: [B=2, S=2048, E=1024] fp32 (key/value args are IGNORED by the
  reference - q,k,v all come from the fused qkv projection of `query`).
  qkv_w: [3E, E] fp32, qkv_b: [3E] fp32.
  out = softmax(10*tanh((q@k^T)/sqrt(64))) @ v, reshaped to [B, S, E].

Sharding: 8 cores, head-parallel. Core c computes heads {2c, 2c+1} for both
batches; qkv weight is column-split (rows of W) to match. Every core reads the
full query; output columns are concatenated on the host.

Math note: softmax(10*tanh(y)) == softmax(20*sigmoid(2y)) exactly (tanh = 2*sigmoid-1
and softmax is shift-invariant), so the two ACT passes are Sigmoid then Exp.
The softmax denominator comes for free from a ones-column appended to v
(matmul M=65), and the normalization happens on the small [S, 64] output.
"""

import os
import numpy as np
from contextlib import ExitStack

import concourse.bass as bass
import concourse.mybir as mybir
import concourse.tile as tile
from concourse import bacc
from concourse.bass_utils import run_bass_kernel_spmd
from concourse.masks import make_identity

# problem constants
B, S, E = 2, 2048, 1024
H = 16
HD = 64
NC = 8
HPC = H // NC            # heads per core = 2
WR = 3 * HPC * HD        # w rows per core = 384
T = B * S                # total tokens = 4096
TCH = 512                # tokens per phase-1 chunk
NCH = T // TCH           # 8 chunks
JT = S // 128            # 16 j tiles per batch
NIC = S // 512           # 4 i-chunks per batch

F32 = mybir.dt.float32
F32R = mybir.dt.float32r
BF16 = mybir.dt.bfloat16

_CACHE = {}


def _build_program():
    nc = bacc.Bacc("TRN2", target_bir_lowering=False, debug=False)

    x_d = nc.dram_tensor("x", [T, E], F32R, kind="ExternalInput").ap()
    wt_d = nc.dram_tensor("wt", [E, WR], F32R, kind="ExternalInput").ap()
    b_d = nc.dram_tensor("b", [WR, 1], F32, kind="ExternalInput").ap()
    out_d = nc.dram_tensor("out", [T, HPC * HD], F32, kind="ExternalOutput").ap()

    with tile.TileContext(nc) as tc, ExitStack() as ctx:
        constp = ctx.enter_context(tc.tile_pool(name="const", bufs=1))
        qkp = ctx.enter_context(tc.tile_pool(name="qk", bufs=1))

        ident_f32 = constp.tile([128, 128], F32)
        make_identity(nc, ident_f32[:])
        ident_r = constp.tile([128, 128], F32R)
        nc.vector.tensor_copy(ident_r[:], ident_f32[:])
        ident = ident_r[:]
        neg10 = constp.tile([128, 1], F32)
        nc.vector.memset(neg10[:], -10.0)
        identf = constp.tile([HD + 1, HD + 1], F32)
        make_identity(nc, identf[:])

        wt_sb = constp.tile([128, 8, WR], F32R)   # [e_part, e_tile, wrow]
        for et in range(8):
            nc.sync.dma_start(wt_sb[:, et, :], wt_d[et * 128:(et + 1) * 128, :])
        bias_sb = constp.tile([128, 3], F32)     # col w = bias for wrow tile w
        for w in range(3):
            nc.sync.dma_start(bias_sb[:, w:w + 1], b_d[w * 128:(w + 1) * 128, :])

        qT = qkp.tile([128, T], F32R)             # [local q wrow (hh*64+d), token]
        kT = qkp.tile([128, T], F32R)             # [local k wrow, token]
        # v natural + ones column: [token%128, token//128, hh, 65]
        v_sb = qkp.tile([128, T // 128, HPC, HD + 1], BF16)
        nc.vector.memset(v_sb[:, :, :, HD:HD + 1], 1.0)

        # ---------------- phase 1: qkv projection ----------------
        with tc.tile_pool(name="xp", bufs=2) as xp, \
             tc.tile_pool(name="xtp", bufs=2) as xtp, \
             tc.tile_pool(name="vtt", bufs=2) as vtt, \
             tc.tile_pool(name="ps_xt", bufs=2, space="PSUM") as ps_xt, \
             tc.tile_pool(name="ps_qkv", bufs=3, space="PSUM") as ps_qkv, \
             tc.tile_pool(name="ps_vt", bufs=2, space="PSUM") as ps_vt:
            for mc in range(NCH):
                x_ch = xp.tile([128, 4, E], F32R)
                for tt in range(4):
                    nc.sync.dma_start(
                        x_ch[:, tt, :],
                        x_d[mc * TCH + tt * 128: mc * TCH + (tt + 1) * 128, :])
                xt_ch = xtp.tile([128, 8, TCH], F32R)   # [e_part, e_tile, token]
                for et in range(8):
                    ps = ps_xt.tile([128, TCH], F32R, tag="psxt")
                    for tt in range(4):
                        nc.tensor.transpose(
                            ps[:, tt * 128:(tt + 1) * 128],
                            x_ch[:, tt, et * 128:(et + 1) * 128],
                            ident)
                    nc.vector.tensor_copy(xt_ch[:, et, :], ps[:])
                # q,k wrow tiles (w=0: q, w=1: k) -> transposed layout directly
                for w in range(2):
                    ps = ps_qkv.tile([128, TCH], F32, tag="psqkv")
                    for et in range(8):
                        nc.tensor.matmul(
                            ps[:], wt_sb[:, et, w * 128:(w + 1) * 128],
                            xt_ch[:, et, :], start=(et == 0), stop=(et == 7))
                    dst = qT if w == 0 else kT
                    nc.vector.tensor_scalar_add(
                        dst[:, mc * TCH:(mc + 1) * TCH], ps[:], bias_sb[:, w:w + 1])
                # v: compute vT then transpose back to natural layout
                ps = ps_qkv.tile([128, TCH], F32, tag="psqkv")
                for et in range(8):
                    nc.tensor.matmul(
                        ps[:], wt_sb[:, et, 2 * 128:3 * 128],
                        xt_ch[:, et, :], start=(et == 0), stop=(et == 7))
                vt_tmp = vtt.tile([128, TCH], F32R)
                nc.vector.tensor_scalar_add(vt_tmp[:], ps[:], bias_sb[:, 2:3])
                for tt in range(4):
                    psv = ps_vt.tile([128, 128], F32R, tag="psvt")
                    nc.tensor.transpose(
                        psv[:], vt_tmp[:, tt * 128:(tt + 1) * 128], ident)
                    # [t, wrow] -> v_sb[t, tc, hh, 0:64] (wrow = hh*64+d)
                    nc.vector.tensor_copy(
                        v_sb[:, mc * 4 + tt, :, 0:HD],
                        psv[:].rearrange("p (hh d) -> p hh d", hh=HPC))

        # ---------------- phase 2: attention ----------------
        with tc.tile_pool(name="ps_s", bufs=1, space="PSUM") as ps_s, \
             tc.tile_pool(name="ps_o", bufs=2, space="PSUM") as ps_o, \
             tc.tile_pool(name="ps_ot", bufs=2, space="PSUM") as ps_ot, \
             tc.tile_pool(name="sigp", bufs=2) as sigp, \
             tc.tile_pool(name="eap", bufs=1) as eap, \
             tc.tile_pool(name="o2p", bufs=2) as o2p, \
             tc.tile_pool(name="oop", bufs=3) as oop, \
             tc.tile_pool(name="rsp", bufs=3) as rsp:
            for b in range(B):
                for hh in range(HPC):
                    hp = hh * HD
                    ea = eap.tile([128, JT, S], BF16, tag="ea")   # [j%128, jt, i]
                    sig = None
                    for jt in range(JT):
                        ps = ps_s.tile([128, S], F32, tag="pss")
                        for ic in range(NIC):
                            nc.tensor.matmul(
                                ps[:, ic * 512:(ic + 1) * 512],
                                kT[hp:hp + HD,
                                   b * S + jt * 128: b * S + (jt + 1) * 128],
                                qT[hp:hp + HD,
                                   b * S + ic * 512: b * S + (ic + 1) * 512],
                                start=True, stop=True)
                        if jt % 2 == 0:
                            sig = sigp.tile([128, 2, S], F32, tag="sig")
                        # p = sigmoid(2 * s_raw / 8) = sigmoid(0.25 * s_raw)
                        nc.scalar.activation(
                            sig[:, jt % 2, :], ps[:],
                            mybir.ActivationFunctionType.Sigmoid, scale=0.25)
                        if jt % 2 == 1:
                            # w = exp(20p - 10)
                            nc.scalar.activation(
                                ea[:, jt - 1:jt + 1, :], sig[:],
                                mybir.ActivationFunctionType.Exp,
                                bias=neg10[:], scale=20.0)
                    for ic in range(NIC):
                        po = ps_o.tile([HD + 1, 512], F32, tag="pso")
                        for jt in range(JT):
                            nc.tensor.matmul(
                                po[:],
                                v_sb[:, b * JT + jt, hh, :],
                                ea[:, jt, ic * 512:(ic + 1) * 512],
                                start=(jt == 0), stop=(jt == JT - 1))
                        o2 = o2p.tile([HD + 1, 512], F32, tag="o2")
                        nc.vector.tensor_copy(o2[:], po[:])
                        oo = oop.tile([128, 4, HD], F32, tag="oo")
                        for tt in range(4):
                            pt = ps_ot.tile([128, HD + 1], F32, tag="psot")
                            nc.tensor.transpose(
                                pt[:], o2[:, tt * 128:(tt + 1) * 128],
                                identf[:])
                            rs = rsp.tile([128, 1], F32, tag="rs")
                            nc.vector.reciprocal(rs[:], pt[:, HD:HD + 1])
                            nc.vector.tensor_scalar_mul(
                                oo[:, tt, :], pt[:, 0:HD], rs[:])
                        nc.sync.dma_start(
                            out_d[b * S + ic * 512: b * S + (ic + 1) * 512, hp:hp + HD]
                            .rearrange("(tt p) d -> p tt d", p=128),
                            oo[:])
    nc.compile()
    return nc


def _get_program():
    if "nc" not in _CACHE:
        _CACHE["nc"] = _build_program()
    return _CACHE["nc"]


def kernel(query, key, value, qkv_w, qkv_b):
    X = np.ascontiguousarray(np.asarray(query, dtype=np.float32).reshape(T, E))
    W = np.asarray(qkv_w, dtype=np.float32)
    bias = np.asarray(qkv_b, dtype=np.float32)

    nc = _get_program()
    in_maps = []
    for c in range(NC):
        rows = []
        for part in range(3):  # q, k, v row blocks of W
            lo = part * E + c * HPC * HD
            rows.append(np.arange(lo, lo + HPC * HD))
        rows = np.concatenate(rows)
        Wc = W[rows, :]                                   # [384, 1024]
        WcT = np.ascontiguousarray(Wc.T)                  # [1024, 384]
        bc = np.ascontiguousarray(bias[rows].reshape(WR, 1))
        in_maps.append({"x": X, "wt": WcT, "b": bc})

    res = run_bass_kernel_spmd(nc, in_maps, list(range(NC)))
    cols = [res.results[c]["out"] for c in range(NC)]     # each [4096, 128]
    full = np.concatenate(cols, axis=1).reshape(B, S, E)
    return np.ascontiguousarray(full.astype(np.float32))


# revision 10
# speedup vs baseline: 134.0928x; 134.0928x over previous
"""Trainium2 Bass kernel for CustomMHA (fused qkv proj + tanh-clipped softmax attn).

Problem (hardcoded shapes):
  query/key/value: [B=2, S=2048, E=1024] fp32 (key/value args are IGNORED by the
  reference - q,k,v all come from the fused qkv projection of `query`).
  qkv_w: [3E, E] fp32, qkv_b: [3E] fp32.
  out = softmax(10*tanh((q@k^T)/sqrt(64))) @ v, reshaped to [B, S, E].

Sharding: 8 cores, head-parallel. Core c computes heads {2c, 2c+1} for both
batches; qkv weight is column-split (rows of W) to match. Every core reads the
full query; output columns are concatenated on the host.

Math note: softmax(10*tanh(y)) == softmax(20*sigmoid(2y)) exactly (tanh = 2*sigmoid-1
and softmax is shift-invariant), so the two ACT passes are Sigmoid then Exp.
The softmax denominator comes for free from a ones-column appended to v
(matmul M=65), and the normalization happens on the small [S, 64] output.
"""

import os
import numpy as np
from contextlib import ExitStack

import concourse.bass as bass
import concourse.mybir as mybir
import concourse.tile as tile
from concourse import bacc
from concourse.bass_utils import run_bass_kernel_spmd
from concourse.masks import make_identity

# problem constants
B, S, E = 2, 2048, 1024
H = 16
HD = 64
NC = 8
HPC = H // NC            # heads per core = 2
WR = 3 * HPC * HD        # w rows per core = 384
T = B * S                # total tokens = 4096
TCH = 512                # tokens per phase-1 chunk
NCH = T // TCH           # 8 chunks
JT = S // 128            # 16 j tiles per batch
NIC = S // 512           # 4 i-chunks per batch

F32 = mybir.dt.float32
F32R = mybir.dt.float32r
BF16 = mybir.dt.bfloat16

_CACHE = {}


def _build_program(repeat=1):
    nc = bacc.Bacc("TRN2", target_bir_lowering=False, debug=False)

    x_d = nc.dram_tensor("x", [T, E], F32R, kind="ExternalInput").ap()
    wt_d = nc.dram_tensor("wt", [E, WR], F32R, kind="ExternalInput").ap()
    b_d = nc.dram_tensor("b", [WR, 1], F32, kind="ExternalInput").ap()
    out_d = nc.dram_tensor("out", [T, HPC * HD], F32, kind="ExternalOutput").ap()

    with tile.TileContext(nc) as tc:
      for _rep in range(repeat):
       with ExitStack() as ctx:
        constp = ctx.enter_context(tc.tile_pool(name=f"const{_rep}", bufs=1))
        qkp = ctx.enter_context(tc.tile_pool(name=f"qk{_rep}", bufs=1))

        ident_f32 = constp.tile([128, 128], F32)
        make_identity(nc, ident_f32[:])
        ident_r = constp.tile([128, 128], F32R)
        nc.vector.tensor_copy(ident_r[:], ident_f32[:])
        ident = ident_r[:]
        neg10 = constp.tile([128, 1], F32)
        nc.vector.memset(neg10[:], -10.0)
        identf = constp.tile([HD + 1, HD + 1], F32)
        make_identity(nc, identf[:])

        wt_sb = constp.tile([128, 8, WR], F32R)   # [e_part, e_tile, wrow]
        for et in range(8):
            nc.sync.dma_start(wt_sb[:, et, :], wt_d[et * 128:(et + 1) * 128, :])
        bias_sb = constp.tile([128, 3], F32)     # col w = bias for wrow tile w
        for w in range(3):
            nc.sync.dma_start(bias_sb[:, w:w + 1], b_d[w * 128:(w + 1) * 128, :])

        qT = qkp.tile([128, T], F32R)             # [local q wrow (hh*64+d), token]
        kT = qkp.tile([128, T], F32R)             # [local k wrow, token]
        # v natural + ones column: [token%128, token//128, hh, 65]
        v_sb = qkp.tile([128, T // 128, HPC, HD + 1], BF16)
        nc.vector.memset(v_sb[:, :, :, HD:HD + 1], 1.0)

        # ---------------- phase 1: qkv projection ----------------
        with tc.tile_pool(name=f"xp{_rep}", bufs=2) as xp, \
             tc.tile_pool(name=f"xtp{_rep}", bufs=2) as xtp, \
             tc.tile_pool(name=f"vtt{_rep}", bufs=2) as vtt, \
             tc.tile_pool(name=f"ps_xt{_rep}", bufs=2, space="PSUM") as ps_xt, \
             tc.tile_pool(name=f"ps_qkv{_rep}", bufs=3, space="PSUM") as ps_qkv, \
             tc.tile_pool(name=f"ps_vt{_rep}", bufs=2, space="PSUM") as ps_vt:
            for mc in range(NCH):
                x_ch = xp.tile([128, 4, E], F32R)
                for tt in range(4):
                    nc.sync.dma_start(
                        x_ch[:, tt, :],
                        x_d[mc * TCH + tt * 128: mc * TCH + (tt + 1) * 128, :])
                xt_ch = xtp.tile([128, 8, TCH], F32R)   # [e_part, e_tile, token]
                for et in range(8):
                    ps = ps_xt.tile([128, TCH], F32R, tag="psxt")
                    for tt in range(4):
                        nc.tensor.transpose(
                            ps[:, tt * 128:(tt + 1) * 128],
                            x_ch[:, tt, et * 128:(et + 1) * 128],
                            ident)
                    nc.vector.tensor_copy(xt_ch[:, et, :], ps[:])
                # q,k wrow tiles (w=0: q, w=1: k) -> transposed layout directly
                for w in range(2):
                    ps = ps_qkv.tile([128, TCH], F32, tag="psqkv")
                    for et in range(8):
                        nc.tensor.matmul(
                            ps[:], wt_sb[:, et, w * 128:(w + 1) * 128],
                            xt_ch[:, et, :], start=(et == 0), stop=(et == 7))
                    dst = qT if w == 0 else kT
                    nc.vector.tensor_scalar_add(
                        dst[:, mc * TCH:(mc + 1) * TCH], ps[:], bias_sb[:, w:w + 1])
                # v: compute vT then transpose back to natural layout
                ps = ps_qkv.tile([128, TCH], F32, tag="psqkv")
                for et in range(8):
                    nc.tensor.matmul(
                        ps[:], wt_sb[:, et, 2 * 128:3 * 128],
                        xt_ch[:, et, :], start=(et == 0), stop=(et == 7))
                vt_tmp = vtt.tile([128, TCH], F32R)
                nc.vector.tensor_scalar_add(vt_tmp[:], ps[:], bias_sb[:, 2:3])
                for tt in range(4):
                    psv = ps_vt.tile([128, 128], F32R, tag="psvt")
                    nc.tensor.transpose(
                        psv[:], vt_tmp[:, tt * 128:(tt + 1) * 128], ident)
                    # [t, wrow] -> v_sb[t, tc, hh, 0:64] (wrow = hh*64+d)
                    nc.vector.tensor_copy(
                        v_sb[:, mc * 4 + tt, :, 0:HD],
                        psv[:].rearrange("p (hh d) -> p hh d", hh=HPC))

        # ---------------- phase 2: attention ----------------
        with tc.tile_pool(name=f"ps_s{_rep}", bufs=1, space="PSUM") as ps_s, \
             tc.tile_pool(name=f"ps_o{_rep}", bufs=2, space="PSUM") as ps_o, \
             tc.tile_pool(name=f"ps_ot{_rep}", bufs=2, space="PSUM") as ps_ot, \
             tc.tile_pool(name=f"sigp{_rep}", bufs=2) as sigp, \
             tc.tile_pool(name=f"eap{_rep}", bufs=1) as eap, \
             tc.tile_pool(name=f"o2p{_rep}", bufs=2) as o2p, \
             tc.tile_pool(name=f"oop{_rep}", bufs=3) as oop, \
             tc.tile_pool(name=f"rsp{_rep}", bufs=3) as rsp:
            for b in range(B):
                for hh in range(HPC):
                    hp = hh * HD
                    ea = eap.tile([128, JT, S], BF16, tag="ea")   # [j%128, jt, i]
                    sig = None
                    for jt in range(JT):
                        ps = ps_s.tile([128, S], F32, tag="pss")
                        for ic in range(NIC):
                            nc.tensor.matmul(
                                ps[:, ic * 512:(ic + 1) * 512],
                                kT[hp:hp + HD,
                                   b * S + jt * 128: b * S + (jt + 1) * 128],
                                qT[hp:hp + HD,
                                   b * S + ic * 512: b * S + (ic + 1) * 512],
                                start=True, stop=True)
                        if jt % 2 == 0:
                            sig = sigp.tile([128, 2, S], F32, tag="sig")
                        # p = sigmoid(2 * s_raw / 8) = sigmoid(0.25 * s_raw)
                        nc.scalar.activation(
                            sig[:, jt % 2, :], ps[:],
                            mybir.ActivationFunctionType.Sigmoid, scale=0.25)
                        if jt % 2 == 1:
                            # w = exp(20p - 10)
                            nc.scalar.activation(
                                ea[:, jt - 1:jt + 1, :], sig[:],
                                mybir.ActivationFunctionType.Exp,
                                bias=neg10[:], scale=20.0)
                    for ic in range(NIC):
                        po = ps_o.tile([HD + 1, 512], F32, tag="pso")
                        for jt in range(JT):
                            nc.tensor.matmul(
                                po[:],
                                v_sb[:, b * JT + jt, hh, :],
                                ea[:, jt, ic * 512:(ic + 1) * 512],
                                start=(jt == 0), stop=(jt == JT - 1))
                        o2 = o2p.tile([HD + 1, 512], F32, tag="o2")
                        nc.vector.tensor_copy(o2[:], po[:])
                        oo = oop.tile([128, 4, HD], F32, tag="oo")
                        for tt in range(4):
                            pt = ps_ot.tile([128, HD + 1], F32, tag="psot")
                            nc.tensor.transpose(
                                pt[:], o2[:, tt * 128:(tt + 1) * 128],
                                identf[:])
                            rs = rsp.tile([128, 1], F32, tag="rs")
                            nc.vector.reciprocal(rs[:], pt[:, HD:HD + 1])
                            nc.vector.tensor_scalar_mul(
                                oo[:, tt, :], pt[:, 0:HD], rs[:])
                        nc.sync.dma_start(
                            out_d[b * S + ic * 512: b * S + (ic + 1) * 512, hp:hp + HD]
                            .rearrange("(tt p) d -> p tt d", p=128),
                            oo[:])
    nc.compile()
    return nc


def _get_program():
    if "nc" not in _CACHE:
        _CACHE["nc"] = _build_program()
    return _CACHE["nc"]


def kernel(query, key, value, qkv_w, qkv_b):
    X = np.ascontiguousarray(np.asarray(query, dtype=np.float32).reshape(T, E))
    W = np.asarray(qkv_w, dtype=np.float32)
    bias = np.asarray(qkv_b, dtype=np.float32)

    nc = _get_program()
    in_maps = []
    for c in range(NC):
        rows = []
        for part in range(3):  # q, k, v row blocks of W
            lo = part * E + c * HPC * HD
            rows.append(np.arange(lo, lo + HPC * HD))
        rows = np.concatenate(rows)
        Wc = W[rows, :]                                   # [384, 1024]
        WcT = np.ascontiguousarray(Wc.T)                  # [1024, 384]
        bc = np.ascontiguousarray(bias[rows].reshape(WR, 1))
        in_maps.append({"x": X, "wt": WcT, "b": bc})

    res = run_bass_kernel_spmd(nc, in_maps, list(range(NC)))
    cols = [res.results[c]["out"] for c in range(NC)]     # each [4096, 128]
    full = np.concatenate(cols, axis=1).reshape(B, S, E)
    return np.ascontiguousarray(full.astype(np.float32))
